# revision 2
# baseline (speedup 1.0000x reference)
"""Causal self-attention on 8 trn2 cores.

Sharding: core = 2*b + g  (b in 0..3 data-parallel over batch,
g in 0..1 tensor-parallel over head groups of 8 heads).
Each core computes, for its (batch, 8 heads):
  Q^T/K^T = W[q|k]^T @ x^T   (feature-major, f32r matmuls)
  V       = x @ Wv           (token-major, f16, with a ones column per head)
  S^T     = K^T_h.T-free matmul (K=64, two heads packed in PE row groups)
  P^T     = exp(S^T/8) in f16, causal mask as 0/1 multiply on diagonal tiles
  O^T_h   = [V_h|1].T @ P^T_h  (f16, row 64 = softmax denominator)
  o_h     = O^T_h * broadcast(1/denom)
  y_part  = O^T @ Wproj[rows of this head group]  (f16 weights)
Host sums the two partial y per batch and adds bproj.

Self-contained: hardcodes B=4, T=2048, C=1024, H=16.
"""
import os
import numpy as np

import concourse.bacc as bacc
import concourse.tile as tile
from concourse import mybir
from concourse import bass_utils
from contextlib import ExitStack

f32 = mybir.dt.float32
f32r = mybir.dt.float32r
f16 = mybir.dt.float16

B, T, C, H = 4, 2048, 1024, 16
HL, D = 8, 64            # local heads per core, head dim
DL = HL * D              # 512 local qkv features
QW = 512                 # q-chunk width
NQC = T // QW            # 4 q chunks
KT = T // 128            # 16 k tiles

_cache = {}


def _build():
    nc = bacc.Bacc(None, target_bir_lowering=False, debug=False)
    with tile.TileContext(nc) as tc, ExitStack() as ctx:
        xT = nc.dram_tensor("xT", [C, T], f32r, kind="ExternalInput")
        wq = nc.dram_tensor("wq", [C, DL], f32r, kind="ExternalInput")
        wk = nc.dram_tensor("wk", [C, DL], f32r, kind="ExternalInput")
        wv = nc.dram_tensor("wv", [C, DL], f32r, kind="ExternalInput")
        wo = nc.dram_tensor("wo", [DL, C], f16, kind="ExternalInput")
        msk = nc.dram_tensor("msk", [128, 4 * QW], f16, kind="ExternalInput")
        y = nc.dram_tensor("y", [T, C], f32, kind="ExternalOutput")

        # ---- all pools in one scope: disjoint PSUM banks / SBUF, no
        # cross-phase WAR serialization ----
        pers = ctx.enter_context(tc.tile_pool(name="pers", bufs=1))
        xpool = ctx.enter_context(tc.tile_pool(name="xpool", bufs=1))
        wpool = ctx.enter_context(tc.tile_pool(name="wpool", bufs=2))
        wvpool = ctx.enter_context(tc.tile_pool(name="wvpool", bufs=1))
        ptp = ctx.enter_context(tc.tile_pool(name="ptp", bufs=2))
        nrm = ctx.enter_context(tc.tile_pool(name="nrm", bufs=1))
        pcs = ctx.enter_context(tc.tile_pool(name="pcs", bufs=4))
        psA = ctx.enter_context(tc.tile_pool(name="psA", bufs=2, space="PSUM"))
        ps_s = ctx.enter_context(tc.tile_pool(name="ps_s", bufs=2, space="PSUM"))

        qsb = [pers.tile([128, T], f16, tag=f"qsb{m}", name=f"qsb{m}") for m in range(4)]
        ksb = [pers.tile([128, T], f16, tag=f"ksb{m}", name=f"ksb{m}") for m in range(4)]
        vsb = [pers.tile([128, HL, D + 1], f16, tag=f"vsb{t}", name=f"vsb{t}") for t in range(KT)]
        osb = [pers.tile([128, T], f16, tag=f"osb{m}", name=f"osb{m}") for m in range(4)]
        mask_sb = pers.tile([128, 4 * QW], f16, tag="mask", name="mask")
        wo_sb = [pers.tile([128, C], f16, tag=f"wo{i}", name=f"wo{i}") for i in range(4)]

        nc.sync.dma_start(out=mask_sb, in_=msk[:, :])
        for i in range(4):
            nc.sync.dma_start(out=wo_sb[i], in_=wo[i * 128:(i + 1) * 128, :])

        # ================= Phase A: QKV projections =================
        xt = []
        for ct in range(8):
            xti = xpool.tile([128, T], f32r, tag=f"xt{ct}", name=f"xt{ct}")
            # gpsimd (SWDGE) queue: keeps the HWDGE queue free for the weight
            # tiles so PE can start as soon as xt[0] + wm[0] land
            nc.gpsimd.dma_start(out=xti, in_=xT[ct * 128:(ct + 1) * 128, :])
            xt.append(xti)

        # Q^T, K^T: feature-major [DL, T], stored f16
        for wdr, dst in ((wq, qsb), (wk, ksb)):
            for m in range(4):
                wm = wpool.tile([128, 8, 128], f32r, tag="wm")
                nc.sync.dma_start(
                    out=wm,
                    in_=wdr[:, m * 128:(m + 1) * 128].rearrange(
                        "(ct p) mc -> p ct mc", p=128))
                for n in range(4):
                    ps = psA.tile([128, 512], f32, tag="ps", name="ps")
                    for ct in range(8):
                        nc.tensor.matmul(ps, wm[:, ct, :],
                                         xt[ct][:, n * 512:(n + 1) * 512],
                                         start=(ct == 0), stop=(ct == 7))
                    nc.vector.tensor_copy(dst[m][:, n * 512:(n + 1) * 512], ps)

        # V: token-major [T, HL, 65] f16, ones col per head
        wvw = wvpool.tile([128, 8, 512], f32r, tag="wvw")
        nc.sync.dma_start(out=wvw, in_=wv[:, :].rearrange(
            "(ct p) f -> p ct f", p=128))
        for t in range(KT):
            ps = psA.tile([128, 512], f32, tag="ps", name="ps")
            for ct in range(8):
                nc.tensor.matmul(ps, xt[ct][:, t * 128:(t + 1) * 128],
                                 wvw[:, ct, :],
                                 start=(ct == 0), stop=(ct == 7))
            nc.vector.tensor_copy(
                vsb[t][:, :, 0:D],
                ps[:].rearrange("p (h d) -> p h d", h=HL))
            nc.vector.memset(vsb[t][:, :, D:D + 1], 1.0)

        # ================= Phase B: attention =================
        for qc in range(NQC):
            blocks = qc + 1
            last_kt = 4 * qc + 3
            for hp in range(4):
                pair = (2 * hp, 2 * hp + 1)
                pvp = {h: psA.tile([D + 1, QW], f32, tag="pv", name="pv")
                       for h in pair}
                ptb = {}

                def emit_pv(blk):
                    for j in range(4):
                        kt = blk * 4 + j
                        for h in pair:
                            nc.tensor.matmul(
                                pvp[h], vsb[kt][:, h, :],
                                ptb[(h, blk)][:, j * 512:(j + 1) * 512],
                                start=(kt == 0), stop=(kt == last_kt))

                for blk in range(blocks):
                    for h in pair:
                        ptb[(h, blk)] = ptp.tile([128, 4 * 512], f16,
                                                 tag=f"pt{h % 2}", name=f"pt{h % 2}")
                    for k2 in range(2):
                        for h in pair:
                            r0 = 64 * (h % 2)
                            ss = ps_s.tile([128, 1024], f32, tag="ss", name="ss")
                            for j in (0, 1):
                                kt = blk * 4 + k2 * 2 + j
                                nc.tensor.matmul(
                                    ss[:, j * 512:(j + 1) * 512],
                                    ksb[h // 2][r0:r0 + 64, kt * 128:(kt + 1) * 128],
                                    qsb[h // 2][r0:r0 + 64, qc * QW:(qc + 1) * QW],
                                    start=True, stop=True)
                            nc.scalar.activation(
                                ptb[(h, blk)][:, k2 * 1024:(k2 + 1) * 1024], ss[:, :],
                                mybir.ActivationFunctionType.Exp, scale=0.125)
                    if blk == blocks - 1:
                        # causal 0/1 mask on the diagonal block
                        for h in pair:
                            for j in range(4):
                                sl = ptb[(h, blk)][:, j * 512:(j + 1) * 512]
                                nc.vector.tensor_mul(
                                    sl, sl, mask_sb[:, j * 512:(j + 1) * 512])
                    if blk > 0:
                        emit_pv(blk - 1)
                emit_pv(blocks - 1)

                # normalize
                for h in pair:
                    f = h // 2
                    dens = nrm.tile([D + 1, QW], f32, tag="dens", name="dens")
                    nc.vector.tensor_copy(dens[D:D + 1, :], pvp[h][D:D + 1, :])
                    den0 = nrm.tile([1, QW], f32, tag="den0", name="den0")
                    nc.sync.dma_start(out=den0[0:1, :], in_=dens[D:D + 1, :])
                    bcd = nrm.tile([D, QW], f32, tag="bcd", name="bcd")
                    nc.gpsimd.partition_broadcast(bcd[:, :], den0[0:1, :])
                    bc = nrm.tile([D, QW], f32, tag="bc", name="bc")
                    nc.vector.reciprocal_approx_fast(out=bc[:, :], in_=bcd[:, :])
                    if h % 2 == 0:
                        nc.vector.tensor_mul(
                            osb[f][0:64, qc * QW:(qc + 1) * QW],
                            pvp[h][0:D, :], bc[:, :])
                    else:
                        tmp = nrm.tile([D, QW], f16, tag="tmp", name="tmp")
                        nc.vector.tensor_mul(tmp[:, :], pvp[h][0:D, :], bc[:, :])
                        nc.sync.dma_start(
                            out=osb[f][64:128, qc * QW:(qc + 1) * QW],
                            in_=tmp[:, :])

        # ================= Phase C: output projection =================
        for t in range(KT):
            for n2 in range(2):
                ps = psA.tile([128, 512], f32, tag="ps", name="ps")
                for m in range(4):
                    nc.tensor.matmul(ps, osb[m][:, t * 128:(t + 1) * 128],
                                     wo_sb[m][:, n2 * 512:(n2 + 1) * 512],
                                     start=(m == 0), stop=(m == 3))
                yt = pcs.tile([128, 512], f32, tag="yt", name="yt")
                nc.vector.tensor_copy(yt, ps)
                nc.sync.dma_start(
                    out=y[t * 128:(t + 1) * 128, n2 * 512:(n2 + 1) * 512],
                    in_=yt)

    nc.compile()
    return nc


def _masks():
    m = np.zeros((128, 4 * QW), dtype=np.float16)
    q = np.arange(QW)
    for j in range(4):
        keep = (j * 128 + np.arange(128))[:, None] <= q[None, :]
        m[:, j * QW:(j + 1) * QW] = keep.astype(np.float16)
    return m


def kernel(x, Wqkv, bqkv, bproj=None, Wproj=None, **kw):
    # tolerate arbitrary kw ordering from harness
    if Wproj is None:
        Wproj = kw["Wproj"]
    x = np.asarray(x, dtype=np.float32)
    Wqkv = np.asarray(Wqkv, dtype=np.float32)
    bqkv = np.asarray(bqkv, dtype=np.float32)
    Wproj = np.asarray(Wproj, dtype=np.float32)
    bproj = np.asarray(bproj, dtype=np.float32)
    assert not np.any(bqkv), "nonzero bqkv not supported by this build"

    if "nc" not in _cache:
        _cache["nc"] = _build()
    nc = _cache["nc"]

    w3 = Wqkv.reshape(C, 3, H, D)
    msk = _masks()
    in_maps = []
    for core in range(8):
        b, g = core // 2, core % 2
        hs = slice(g * HL, (g + 1) * HL)
        in_maps.append({
            "xT": np.ascontiguousarray(x[b].T),
            "wq": np.ascontiguousarray(w3[:, 0, hs, :].reshape(C, DL)),
            "wk": np.ascontiguousarray(w3[:, 1, hs, :].reshape(C, DL)),
            "wv": np.ascontiguousarray(w3[:, 2, hs, :].reshape(C, DL)),
            "wo": np.ascontiguousarray(Wproj[g * DL:(g + 1) * DL, :]).astype(np.float16),
            "msk": msk,
        })

    trace = bool(int(os.environ.get("KERNEL_TRACE", "0")))
    res = bass_utils.run_bass_kernel_spmd(nc, in_maps, core_ids=list(range(8)),
                                          trace=trace)
    _cache["last_exec_ns"] = res.exec_time_ns
    _cache["res"] = res
    if trace:
        print("HW exec time:", res.exec_time_ns, "ns")

    out = np.empty((B, T, C), dtype=np.float32)
    for b in range(B):
        out[b] = res.results[2 * b]["y"] + res.results[2 * b + 1]["y"]
    out += bproj[None, None, :]
    return out



# revision 5
# speedup vs baseline: 1.1816x; 1.1816x over previous
"""Causal self-attention on 8 trn2 cores.

Sharding: core = 2*b + g  (b in 0..3 data-parallel over batch,
g in 0..1 tensor-parallel over head groups of 8 heads).

Per-core pipeline (all f16 matmul operands, f32 psum):
  K^T = Wk^T x^T (feature-major), V = x Wv (token-major, ones col per
  head), Q^T per 512-query chunk.  Attention per (q-chunk, head-pair):
  S^T tiles via row-group-packed K=64 matmuls; softmax exp split across
  engines: off-diagonal blocks on ScalarE (ACT exp), diagonal blocks on
  VectorE via a fused Schraudolph f16-exp (bits = s*C + maskbias ->
  int16, saturating; causal mask folded in as -1e6 bias -> -0.0).
  O^T accumulates [V|1].T @ P^T; softmax denominator rides as psum row
  64.  Normalize: ACT copies den/64 to f16, PE ones-broadcast matmul,
  DVE reciprocal + scaled multiply.  Output projection per q-chunk.
  Emission interleaves QKV/proj work into the attention stream so the
  PE never idles while ACT/DVE run softmax.

Host sums the two tensor-parallel partial y per batch and adds bproj.
Self-contained: hardcodes B=4, T=2048, C=1024, H=16.
"""
import os
import numpy as np

import concourse.bacc as bacc
import concourse.tile as tile
from concourse import mybir
from concourse import bass_utils
from contextlib import ExitStack

f32 = mybir.dt.float32
f32r = mybir.dt.float32r
f16 = mybir.dt.float16
i16 = mybir.dt.int16

B, T, C, H = 4, 2048, 1024, 16
HL, D = 8, 64            # local heads per core, head dim
DL = HL * D              # 512 local qkv features
QW = 512                 # q-chunk width
NQC = T // QW            # 4 q chunks
KT = T // 128            # 16 k tiles
CT = C // 128            # 8 contraction tiles

C_SCHR = 1477.3196 * 0.125   # schraudolph scale (f16 bits per unit score)
B_SCHR = 15316.0             # schraudolph bias (minimax)

_cache = {}


def _build():
    nc = bacc.Bacc(None, target_bir_lowering=False, debug=False)
    with tile.TileContext(nc) as tc, ExitStack() as ctx:
        xT = nc.dram_tensor("xT", [C, T], f16, kind="ExternalInput")
        wq = nc.dram_tensor("wq", [C, DL], f16, kind="ExternalInput")
        wk = nc.dram_tensor("wk", [C, DL], f16, kind="ExternalInput")
        wv = nc.dram_tensor("wv", [C, DL], f16, kind="ExternalInput")
        wo = nc.dram_tensor("wo", [DL, C], f16, kind="ExternalInput")
        mb = nc.dram_tensor("mb", [256, 1024], f32, kind="ExternalInput")
        y = nc.dram_tensor("y", [T, C], f32, kind="ExternalOutput")

        pers = ctx.enter_context(tc.tile_pool(name="pers", bufs=1))
        ptp = ctx.enter_context(tc.tile_pool(name="ptp", bufs=3))
        nrm = ctx.enter_context(tc.tile_pool(name="nrm", bufs=2))
        pcs = ctx.enter_context(tc.tile_pool(name="pcs", bufs=4))
        psA = ctx.enter_context(tc.tile_pool(name="psA", bufs=2, space="PSUM"))
        psS = ctx.enter_context(tc.tile_pool(name="psS", bufs=2, space="PSUM"))

        xt = [pers.tile([128, T], f16, tag=f"xt{ct}", name=f"xt{ct}")
              for ct in range(CT)]
        qsb = [pers.tile([128, T], f16, tag=f"qsb{m}", name=f"qsb{m}") for m in range(4)]
        ksb = [pers.tile([128, T], f16, tag=f"ksb{m}", name=f"ksb{m}") for m in range(4)]
        osb = [pers.tile([128, T], f16, tag=f"osb{m}", name=f"osb{m}") for m in range(4)]
        vsb = [pers.tile([128, HL, D + 1], f16, tag=f"vsb{t}", name=f"vsb{t}")
               for t in range(KT)]
        wo_sb = [pers.tile([128, C], f16, tag=f"wo{i}", name=f"wo{i}") for i in range(4)]
        wkt = pers.tile([128, 4, CT, 128], f16, tag="wkt", name="wkt")
        wqt = pers.tile([128, 4, CT, 128], f16, tag="wqt", name="wqt")
        wvt = pers.tile([128, CT, DL], f16, tag="wvt", name="wvt")
        mb_sb = [pers.tile([128, 1024], f32, tag=f"mb{k2}", name=f"mb{k2}")
                 for k2 in range(2)]
        ones64 = pers.tile([65, 64], f16, tag="ones64", name="ones64")

        # ---- input DMAs: weights on sync queue, x on gpsimd queue ----
        for k2 in range(2):
            nc.sync.dma_start(out=mb_sb[k2], in_=mb[k2 * 128:(k2 + 1) * 128, :])
        for m in range(4):
            nc.sync.dma_start(
                out=wkt[:, m],
                in_=wk[:, m * 128:(m + 1) * 128].rearrange(
                    "(ct p) mc -> p ct mc", p=128))
        nc.sync.dma_start(out=wvt, in_=wv[:, :].rearrange(
            "(ct p) f -> p ct f", p=128))
        for m in range(4):
            nc.sync.dma_start(
                out=wqt[:, m],
                in_=wq[:, m * 128:(m + 1) * 128].rearrange(
                    "(ct p) mc -> p ct mc", p=128))
        for i in range(4):
            nc.sync.dma_start(out=wo_sb[i], in_=wo[i * 128:(i + 1) * 128, :])
        for ct in range(CT):
            nc.gpsimd.dma_start(out=xt[ct], in_=xT[ct * 128:(ct + 1) * 128, :])
        nc.vector.memset(ones64, 1.0)

        # ---- phase-A / phase-C work units ----
        def k_group(n, m):
            ps = psA.tile([128, 512], f32, tag="ps", name="ps")
            for ct in range(CT):
                nc.tensor.matmul(ps, wkt[:, m, ct],
                                 xt[ct][:, n * 512:(n + 1) * 512],
                                 start=(ct == 0), stop=(ct == CT - 1))
            nc.any.tensor_copy(ksb[m][:, n * 512:(n + 1) * 512], ps)

        def q_group(qc, m):
            ps = psA.tile([128, 512], f32, tag="ps", name="ps")
            for ct in range(CT):
                nc.tensor.matmul(ps, wqt[:, m, ct],
                                 xt[ct][:, qc * 512:(qc + 1) * 512],
                                 start=(ct == 0), stop=(ct == CT - 1))
            nc.any.tensor_copy(qsb[m][:, qc * 512:(qc + 1) * 512], ps)

        def v_group(t):
            ps = psA.tile([128, 512], f32, tag="ps", name="ps")
            for ct in range(CT):
                nc.tensor.matmul(ps, xt[ct][:, t * 128:(t + 1) * 128],
                                 wvt[:, ct, :],
                                 start=(ct == 0), stop=(ct == CT - 1))
            nc.any.tensor_copy(
                vsb[t][:, :, 0:D],
                ps[:].rearrange("p (h d) -> p h d", h=HL))
            nc.any.memset(vsb[t][:, :, D:D + 1], 1.0)

        def c_group(t, n2):
            ps = psA.tile([128, 512], f32, tag="ps", name="ps")
            for m in range(4):
                nc.tensor.matmul(ps, osb[m][:, t * 128:(t + 1) * 128],
                                 wo_sb[m][:, n2 * 512:(n2 + 1) * 512],
                                 start=(m == 0), stop=(m == 3))
            yt = pcs.tile([128, 512], f32, tag="yt", name="yt")
            nc.any.tensor_copy(yt, ps)
            nc.sync.dma_start(
                out=y[t * 128:(t + 1) * 128, n2 * 512:(n2 + 1) * 512],
                in_=yt)

        # ---- attention unit for (qc, hp) ----
        def b_unit(qc, hp):
            pair = (2 * hp, 2 * hp + 1)
            last_kt = 4 * qc + 3
            pvp = {h: psA.tile([D + 1, QW], f32, tag="pv", name="pv")
                   for h in pair}
            ptb = {}

            def emit_pv(blk):
                for j in range(4):
                    kt = blk * 4 + j
                    for h in pair:
                        nc.tensor.matmul(
                            pvp[h], vsb[kt][:, h, :],
                            ptb[(h, blk)][:, j * 512:(j + 1) * 512],
                            start=(kt == 0), stop=(kt == last_kt))

            for blk in range(qc + 1):
                diag = blk == qc
                for h in pair:
                    ptb[(h, blk)] = ptp.tile([128, 4 * 512], f16,
                                             tag=f"pt{h % 2}", name=f"pt{h % 2}")
                for k2 in range(2):
                    for h in pair:
                        r0 = 64 * (h % 2)
                        ss = psS.tile([128, 1024], f32, tag="ss", name="ss")
                        for j in (0, 1):
                            kt = blk * 4 + k2 * 2 + j
                            nc.tensor.matmul(
                                ss[:, j * 512:(j + 1) * 512],
                                ksb[hp][r0:r0 + 64, kt * 128:(kt + 1) * 128],
                                qsb[hp][r0:r0 + 64, qc * QW:(qc + 1) * QW],
                                start=True, stop=True)
                        dst = ptb[(h, blk)][:, k2 * 1024:(k2 + 1) * 1024]
                        if diag:
                            # fused schraudolph exp + causal mask on DVE
                            nc.vector.scalar_tensor_tensor(
                                dst.bitcast(i16), ss, C_SCHR, mb_sb[k2],
                                mybir.AluOpType.mult, mybir.AluOpType.add)
                        else:
                            nc.scalar.activation(
                                dst, ss,
                                mybir.ActivationFunctionType.Exp, scale=0.125)
                if blk > 0:
                    emit_pv(blk - 1)
            emit_pv(qc)

            # normalize: den/64 -> f16 -> PE broadcast -> recip -> scaled mul
            for h in pair:
                f = hp
                denf = nrm.tile([1, QW], f16, tag="denf", name="denf")
                nc.scalar.mul(denf, pvp[h][D:D + 1, :], 1.0 / 64.0)
                bc = psA.tile([128, QW], f32, tag="ps", name="ps")
                nc.tensor.matmul(bc[0:64, :], ones64, denf,
                                 start=True, stop=True)
                rcp = nrm.tile([64, QW], f32, tag="rcp", name="rcp")
                nc.vector.reciprocal_approx_fast(out=rcp, in_=bc[0:64, :])
                if h % 2 == 0:
                    nc.vector.scalar_tensor_tensor(
                        osb[f][0:64, qc * QW:(qc + 1) * QW],
                        pvp[h][0:D, :], 1.0 / 64.0, rcp,
                        mybir.AluOpType.mult, mybir.AluOpType.mult)
                else:
                    tmp = nrm.tile([64, QW], f16, tag="tmp", name="tmp")
                    nc.vector.scalar_tensor_tensor(
                        tmp, pvp[h][0:D, :], 1.0 / 64.0, rcp,
                        mybir.AluOpType.mult, mybir.AluOpType.mult)
                    nc.sync.dma_start(
                        out=osb[f][64:128, qc * QW:(qc + 1) * QW],
                        in_=tmp)

        # ---- emission: head, then interleaved attention + filler ----
        for m in range(4):
            k_group(0, m)
        for t in range(4):
            v_group(t)
        for m in range(4):
            q_group(0, m)

        def fillers(qc, hp):
            out = []
            if qc == 0:
                if hp < 3:
                    out += [(k_group, (hp + 1, m)) for m in range(4)]
                    if hp == 2:
                        out += [(v_group, (4,))]
                else:
                    out += [(v_group, (t,)) for t in (5, 6, 7)]
                    out += [(q_group, (1, m)) for m in range(4)]
            elif qc in (1, 2):
                t_v = 4 * (qc + 1) + hp
                out += [(v_group, (t_v,))]
                t_c = 4 * (qc - 1) + hp
                out += [(c_group, (t_c, 0)), (c_group, (t_c, 1))]
                if hp == 3:
                    out += [(q_group, (qc + 1, m)) for m in range(4)]
            else:
                t_c = 8 + hp
                out += [(c_group, (t_c, 0)), (c_group, (t_c, 1))]
            return out

        for qc in range(NQC):
            for hp in range(4):
                b_unit(qc, hp)
                for fn, args in fillers(qc, hp):
                    fn(*args)
        for t in range(12, 16):
            for n2 in range(2):
                c_group(t, n2)

    nc.compile()
    return nc


def _maskbias():
    mb = np.full((2, 128, 1024), B_SCHR, dtype=np.float32)
    kp = np.arange(128)[:, None]
    q = np.arange(512)[None, :]
    for k2 in range(2):
        for j in range(2):
            kt = k2 * 2 + j
            masked = q < (kt * 128 + kp)
            blkv = mb[k2][:, j * 512:(j + 1) * 512]
            blkv[masked] = -1e6
    return mb.reshape(256, 1024)


def kernel(x, Wqkv, bqkv, bproj=None, Wproj=None, **kw):
    # tolerate arbitrary kw ordering from harness
    if Wproj is None:
        Wproj = kw["Wproj"]
    x = np.asarray(x, dtype=np.float32)
    Wqkv = np.asarray(Wqkv, dtype=np.float32)
    bqkv = np.asarray(bqkv, dtype=np.float32)
    Wproj = np.asarray(Wproj, dtype=np.float32)
    bproj = np.asarray(bproj, dtype=np.float32)
    assert not np.any(bqkv), "nonzero bqkv not supported by this build"

    if "nc" not in _cache:
        _cache["nc"] = _build()
    nc = _cache["nc"]

    w3 = Wqkv.reshape(C, 3, H, D)
    mbias = _maskbias()
    in_maps = []
    for core in range(8):
        b, g = core // 2, core % 2
        hs = slice(g * HL, (g + 1) * HL)
        in_maps.append({
            "xT": np.ascontiguousarray(x[b].T).astype(np.float16),
            "wq": np.ascontiguousarray(
                w3[:, 0, hs, :].reshape(C, DL)).astype(np.float16),
            "wk": np.ascontiguousarray(
                w3[:, 1, hs, :].reshape(C, DL)).astype(np.float16),
            "wv": np.ascontiguousarray(
                w3[:, 2, hs, :].reshape(C, DL)).astype(np.float16),
            "wo": np.ascontiguousarray(
                Wproj[g * DL:(g + 1) * DL, :]).astype(np.float16),
            "mb": mbias,
        })

    trace = bool(int(os.environ.get("KERNEL_TRACE", "0")))
    res = bass_utils.run_bass_kernel_spmd(nc, in_maps, core_ids=list(range(8)),
                                          trace=trace)
    _cache["last_exec_ns"] = res.exec_time_ns
    _cache["res"] = res
    if trace:
        print("HW exec time:", res.exec_time_ns, "ns")

    out = np.empty((B, T, C), dtype=np.float32)
    for b in range(B):
        out[b] = res.results[2 * b]["y"] + res.results[2 * b + 1]["y"]
    out += bproj[None, None, :]
    return out
